# revision 12
# baseline (speedup 1.0000x reference)
"""BoundaryMaxPooling Trainium2 kernel, v7: half-time d=8 packed tables.

ap_gather cost is per-index and nearly flat in d (d=8 is only ~14%/idx
over d=4 while fetching 2x the data).  v7 therefore splits the TIME axis
in half (with a 257-wide halo, the max segment length) so SIXTEEN
half-tiles share one index stream, packed as 16 bf16 lanes per 32-byte
entry (d=8).  Each query belongs to exactly one half (by its lo), so the
per-core index count halves vs v4-v6.

Sharding: 8 cores = 2 families x 2 time-halves x 2 batch-groups.
Core c: family f=c//4 (start/end segments), half h=(c//2)%2 (data window
[767h, 767h+1281)), batch group i=c%2 (batches 4i..4i+3).  16 lanes =
4 batches x 4 channel-blocks of that family.

All 9 levels live in ONE tile pool of 4 rotating 41-KB buffers:
L0->b0, L1->b1, L2->b2, L3->b3, L4->b0, L5->b1, L6->b2, L7->b3, L8->b0.
Level k+4 overwrites level k's buffer, which class-k gathers read; since
gather SOURCE reads carry no completion semantics, each of L5..L8 is
preceded by a 2-column copy of class-k's last gather OUTPUT into the
target buffer -- the WAW edge with the build serializes it correctly
(with k<4 host-side no rotated-over buffer is ever read by a gather,
so no guard fires in practice).

Short queries (k<4: len<16, ~5%) and the rare k=8/k=0 queries are
computed on the host (~6% of output) -- their device cost is pure
per-instruction overhead crawling under the build chain.  Each chunk's
index stream is [all probe-1 | all probe-2]; the pairwise
max runs in place on the gather output.  Host packs bf16 input,
deinterleaves/upcasts/un-permutes the output.
"""

import numpy as np

B, C2, T = 8, 1024, 2048
P = 128
N_CORES = 8
KLEV = 9
HT = 1281                                # half window: 1024 + 257 halo
H_OFF = [0, 767]                         # window start per half
HN = [HT + 1 - (1 << k) for k in range(KLEV)]

MAXCH = 320                              # max queries per gather chunk
GROUPS = [5, 6, 7]                      # k<5 and k=8 are host-side

_CACHE = {}


def _plan_chunks(counts):
    chunks = []
    for g in GROUPS:
        n = counts[g]
        while n > 0:
            c = min(n, MAXCH)
            chunks.append((g, ((c + 15) // 16) * 16))
            n -= c
    return chunks


def _build_program(chunks):
    import concourse.bacc as bacc
    import concourse.mybir as mybir
    import concourse.tile as tile

    bf16 = mybir.dt.bfloat16
    i16 = mybir.dt.int16
    i32 = mybir.dt.int32
    MAX = mybir.AluOpType.max

    qtot = sum(c for _, c in chunks)
    idxcols = 2 * qtot // 16

    nc = bacc.Bacc("TRN2", target_bir_lowering=False, debug=False,
                   num_devices=N_CORES)
    feat = nc.dram_tensor("feat", [P, 8 * HT], i32, kind="ExternalInput")
    idxw = nc.dram_tensor("idxw", [P, idxcols], i16, kind="ExternalInput")
    out = nc.dram_tensor("out", [P, 8 * qtot], i32, kind="ExternalOutput")

    with tile.TileContext(nc) as tc:
        with tc.tile_pool(name="tabp", bufs=4) as tp, \
             tc.tile_pool(name="gp", bufs=2) as gp, \
             tc.tile_pool(name="ip", bufs=1) as ip:
            idxt = ip.tile([P, idxcols], i16, tag="idx")
            nc.sync.dma_start(idxt[:], idxw[:])

            lev = [None] * KLEV
            lev[0] = tp.tile([P, 8 * HT], i32, tag="t", name="lev0")
            DB = [0, 320, 640, 960, HT]      # input DMA chunk boundaries
            for a, b in zip(DB, DB[1:]):
                nc.sync.dma_start(lev[0][:, 8 * a:8 * b],
                                  feat[:, 8 * a:8 * b])

            state = {"col": 0, "ocol": 0, "last": {}, "pend": []}

            def do_gathers(k):
                src = lev[k]
                for g, ch in chunks:
                    if g != k:
                        continue
                    ni = 2 * ch
                    gt = gp.tile([P, 16 * MAXCH], i32, tag="g")
                    nc.gpsimd.ap_gather(
                        gt[:, 0:8 * ni], src[:, 0:8 * HN[k]],
                        idxt[:, state["col"]:state["col"] + ni // 16],
                        channels=P, num_elems=HN[k], d=8, num_idxs=ni)
                    state["last"][k] = gt
                    state["pend"].append((gt, ch, state["ocol"]))
                    state["col"] += ni // 16
                    state["ocol"] += 8 * ch

            def flush_finals():
                for gt, ch, ocol in state["pend"]:
                    gb = gt[:].bitcast(bf16)
                    nc.vector.tensor_tensor(
                        gb[:, 0:16 * ch],
                        gb[:, 0:16 * ch], gb[:, 16 * ch:32 * ch], MAX)
                    nc.sync.dma_start(out[:, ocol:ocol + 8 * ch],
                                      gt[:, 0:8 * ch])
                state["pend"] = []

            for k in range(1, 8):
                lev[k] = tp.tile([P, 8 * HT], i32, tag="t",
                                 name=f"lev{k}")
                if k >= 5:
                    # guard: class k-4 gathers read the buffer this level
                    # overwrites; chain through their gather OUTPUT
                    ga = state["last"].get(k - 4)
                    if ga is not None:
                        nc.vector.tensor_copy(lev[k][:, 0:2], ga[:, 0:2])
                s = 1 << (k - 1)
                vo = lev[k][:].bitcast(bf16)
                vi = lev[k - 1][:].bitcast(bf16)
                # wavefront: each level builds in 4 pieces; piece j only
                # needs the previous level's pieces j and j+1 (range-level
                # dependency tracking), so levels pipeline diagonally
                QB = ([0, 318, 638, 958, HN[1]] if k == 1
                      else [0, 320, 640, 960, HN[k]])
                for a, b in zip(QB, QB[1:]):
                    nc.vector.tensor_tensor(
                        vo[:, 16 * a:16 * b],
                        vi[:, 16 * a:16 * b],
                        vi[:, 16 * (a + s):16 * (b + s)], MAX)
                do_gathers(k)
                if k >= 6:
                    # finals for the previous class, issued after this
                    # level's build pieces so builds never queue behind them
                    flush_finals()
            flush_finals()
    nc.compile()
    return nc


def _f32_to_bf16_u16(x):
    u = x.astype(np.float32).view(np.uint32)
    rounded = u + 0x7FFF + ((u >> 16) & 1)
    return (rounded >> 16).astype(np.uint16)


def _queries(segments, max_len):
    seg = np.clip(np.asarray(segments, np.float32)[0], 0.0,
                  np.float32(max_len - 1))
    fams = []
    for f in (0, 1):
        lo = np.floor(seg[:, 2 * f]).astype(np.int64)
        hi = np.ceil(seg[:, 2 * f + 1]).astype(np.int64)
        hi = np.maximum(hi, lo + 1)
        ln = hi - lo
        k = np.floor(np.log2(ln.astype(np.float64))).astype(np.int64)
        fams.append((k, lo, hi - (1 << k)))
    return fams


def _layout(fams):
    """Chunk layout unified across the four (family, half) streams."""
    streams = []
    for f in (0, 1):
        k, p1, p2 = fams[f]
        for h in (0, 1):
            m = (k >= 5) & (k <= 7) & ((p1 >= 1024) == bool(h))
            streams.append((f, h, k, p1, p2, m))
    counts = {g: 0 for g in GROUPS}
    for _, _, k, _, _, m in streams:
        for g in GROUPS:
            counts[g] = max(counts[g], int(np.sum(m & (k == g))))
    chunks = _plan_chunks(counts)
    qtot = sum(c for _, c in chunks)

    lay = {}
    for f, h, k, p1, p2, m in streams:
        tsort = {g: np.nonzero(m & (k == g))[0] for g in GROUPS}
        used = {g: 0 for g in GROUPS}
        idx_stream, perm = [], []
        for g, ch in chunks:
            ts = tsort[g][used[g]:used[g] + ch]
            used[g] += ch
            npad = ch - len(ts)
            a = np.concatenate([p1[ts] - H_OFF[h],
                                np.zeros(npad, np.int64)])
            b = np.concatenate([p2[ts] - H_OFF[h],
                                np.zeros(npad, np.int64)])
            assert (a >= 0).all() and (b >= 0).all()
            assert (a < HN[g]).all() if len(ts) else True
            idx_stream.append(np.concatenate([a, b]))
            perm.append(np.concatenate([ts, -np.ones(npad, np.int64)]))
        idx = np.concatenate(idx_stream).astype(np.int16)
        assert idx.size == 2 * qtot
        wrapped = np.tile(idx.reshape(-1, 16).T, (8, 1)).astype(np.int16)
        lay[(f, h)] = (wrapped, np.concatenate(perm))
    k0s = []
    for f in (0, 1):
        k, p1, p2 = fams[f]
        th = np.nonzero((k < 5) | (k == 8))[0]
        # reconstruct hi = p2 + 2^k
        k0s.append((th, p1[th], p2[th] + (1 << k[th].astype(np.int64))))
    return chunks, lay, k0s


def prepare(feature, segments, max_len):
    feature = np.asarray(feature, np.float32)
    u16 = _f32_to_bf16_u16(feature)           # [B, C2, T]
    fams = _queries(segments, int(max_len))
    chunks, lay, k0s = _layout(fams)
    in_maps, perms = [], []
    for c in range(N_CORES):
        f, h, i = c // 4, (c // 2) % 2, c % 2
        # lanes j = 0..15: batch 4i + j//4, channels 512f + 128*(j%4) + p
        x = u16[4 * i:4 * i + 4, 512 * f:512 * (f + 1),
                H_OFF[h]:H_OFF[h] + HT]                    # [4,512,HT]
        x = x.reshape(4, 4, P, HT).transpose(2, 3, 0, 1)   # [p,e,b,cb]
        packed = np.ascontiguousarray(x.reshape(P, HT, 16)).view(np.uint32)
        packed = packed.reshape(P, 8 * HT).astype(np.int32, copy=False)
        wrapped, perm = lay[(f, h)]
        in_maps.append({"feat": packed, "idxw": wrapped})
        perms.append(perm)
    return chunks, in_maps, perms, k0s


def postprocess(results, perms, k0s, feature):
    feature = np.asarray(feature, np.float32)
    out = np.empty((B, C2, T), np.float32)
    for c in range(N_CORES):
        f, h, i = c // 4, (c // 2) % 2, c % 2
        r = np.asarray(results[c]["out"])          # [P, 8*qtot] i32
        qtot = r.shape[1] // 8
        u16 = r.view(np.uint16).reshape(P, qtot, 16)
        perm = perms[c]
        valid = perm >= 0
        tq = perm[valid]
        v = u16[:, valid, :]                       # [P, nq, 16]
        f32 = (v.astype(np.uint32) << 16).view(np.float32)
        f32 = f32.transpose(2, 0, 1).reshape(4, 4, P, -1).reshape(4, 512, -1)
        out[4 * i:4 * i + 4, 512 * f:512 * (f + 1), :][:, :, tq] = f32
    for f in (0, 1):
        th, loh, hih = k0s[f]
        ch = slice(512 * f, 512 * (f + 1))
        for t, lo, hi in zip(th, loh, hih):
            out[:, ch, t] = feature[:, ch, lo:hi].max(axis=-1)
    return out


def kernel(feature, segments, max_len=T, **_unused):
    from concourse import bass_utils

    feature = np.asarray(feature, dtype=np.float32)
    assert feature.shape == (B, C2, T), feature.shape
    chunks, in_maps, perms, k0s = prepare(feature, segments, int(max_len))

    key = tuple(chunks)
    if _CACHE.get("key") != key:
        _CACHE["nc"] = _build_program(chunks)
        _CACHE["key"] = key
    nc = _CACHE["nc"]

    res = bass_utils.run_bass_kernel_spmd(
        nc, in_maps, core_ids=list(range(N_CORES)))
    return postprocess(res.results, perms, k0s, feature)


# revision 13
# speedup vs baseline: 1.0074x; 1.0074x over previous
"""BoundaryMaxPooling Trainium2 kernel, v7: half-time d=8 packed tables.

ap_gather cost is per-index and nearly flat in d (d=8 is only ~14%/idx
over d=4 while fetching 2x the data).  v7 therefore splits the TIME axis
in half (with a 257-wide halo, the max segment length) so SIXTEEN
half-tiles share one index stream, packed as 16 bf16 lanes per 32-byte
entry (d=8).  Each query belongs to exactly one half (by its lo), so the
per-core index count halves vs v4-v6.

Sharding: 8 cores = 2 families x 2 time-halves x 2 batch-groups.
Core c: family f=c//4 (start/end segments), half h=(c//2)%2 (data window
[767h, 767h+1281)), batch group i=c%2 (batches 4i..4i+3).  16 lanes =
4 batches x 4 channel-blocks of that family.

All 9 levels live in ONE tile pool of 4 rotating 41-KB buffers:
L0->b0, L1->b1, L2->b2, L3->b3, L4->b0, L5->b1, L6->b2, L7->b3, L8->b0.
Level k+4 overwrites level k's buffer, which class-k gathers read; since
gather SOURCE reads carry no completion semantics, each of L5..L8 is
preceded by a 2-column copy of class-k's last gather OUTPUT into the
target buffer -- the WAW edge with the build serializes it correctly
(with k<5 host-side no rotated-over buffer is ever read by a gather,
so no guard fires in practice).

Short queries (k<5: len<32) and the rare k=8/k=0 queries are
computed on the host (~13% of output) -- their device cost is pure
per-instruction overhead crawling under the build chain.  Each chunk's
index stream is [all probe-1 | all probe-2]; the pairwise
max runs in place on the gather output.  Host packs bf16 input,
deinterleaves/upcasts/un-permutes the output.
"""

import numpy as np

B, C2, T = 8, 1024, 2048
P = 128
N_CORES = 8
KLEV = 9
HT = 1281                                # half window: 1024 + 257 halo
H_OFF = [0, 767]                         # window start per half
HN = [HT + 1 - (1 << k) for k in range(KLEV)]

MAXCH = 320                              # max queries per gather chunk
GROUPS = [5, 6, 7]                      # k<5 and k=8 are host-side

_CACHE = {}


def _plan_chunks(counts):
    chunks = []
    for g in GROUPS:
        n = counts[g]
        while n > 0:
            c = min(n, MAXCH)
            chunks.append((g, ((c + 15) // 16) * 16))
            n -= c
    return chunks


def _build_program(chunks):
    import concourse.bacc as bacc
    import concourse.mybir as mybir
    import concourse.tile as tile

    bf16 = mybir.dt.bfloat16
    i16 = mybir.dt.int16
    i32 = mybir.dt.int32
    MAX = mybir.AluOpType.max

    qtot = sum(c for _, c in chunks)
    idxcols = 2 * qtot // 16

    nc = bacc.Bacc("TRN2", target_bir_lowering=False, debug=False,
                   num_devices=N_CORES)
    feat = nc.dram_tensor("feat", [P, 8 * HT], i32, kind="ExternalInput")
    idxw = nc.dram_tensor("idxw", [P, idxcols], i16, kind="ExternalInput")
    out = nc.dram_tensor("out", [P, 8 * qtot], i32, kind="ExternalOutput")

    with tile.TileContext(nc) as tc:
        with tc.tile_pool(name="tabp", bufs=4) as tp, \
             tc.tile_pool(name="gp", bufs=2) as gp, \
             tc.tile_pool(name="ip", bufs=1) as ip:
            idxt = ip.tile([P, idxcols], i16, tag="idx")
            nc.sync.dma_start(idxt[:], idxw[:])

            lev = [None] * KLEV
            lev[0] = tp.tile([P, 8 * HT], i32, tag="t", name="lev0")
            DB = [0, 320, 640, 960, HT]      # input DMA chunk boundaries
            for a, b in zip(DB, DB[1:]):
                nc.sync.dma_start(lev[0][:, 8 * a:8 * b],
                                  feat[:, 8 * a:8 * b])

            state = {"col": 0, "ocol": 0, "last": {}, "pend": []}

            def do_gathers(k):
                src = lev[k]
                for g, ch in chunks:
                    if g != k:
                        continue
                    ni = 2 * ch
                    gt = gp.tile([P, 16 * MAXCH], i32, tag="g")
                    nc.gpsimd.ap_gather(
                        gt[:, 0:8 * ni], src[:, 0:8 * HN[k]],
                        idxt[:, state["col"]:state["col"] + ni // 16],
                        channels=P, num_elems=HN[k], d=8, num_idxs=ni)
                    state["last"][k] = gt
                    state["pend"].append((gt, ch, state["ocol"]))
                    state["col"] += ni // 16
                    state["ocol"] += 8 * ch

            def flush_finals():
                for gt, ch, ocol in state["pend"]:
                    gb = gt[:].bitcast(bf16)
                    nc.vector.tensor_tensor(
                        gb[:, 0:16 * ch],
                        gb[:, 0:16 * ch], gb[:, 16 * ch:32 * ch], MAX)
                    nc.sync.dma_start(out[:, ocol:ocol + 8 * ch],
                                      gt[:, 0:8 * ch])
                state["pend"] = []

            for k in range(1, 8):
                lev[k] = tp.tile([P, 8 * HT], i32, tag="t",
                                 name=f"lev{k}")
                if k >= 5:
                    # guard: class k-4 gathers read the buffer this level
                    # overwrites; chain through their gather OUTPUT
                    ga = state["last"].get(k - 4)
                    if ga is not None:
                        nc.vector.tensor_copy(lev[k][:, 0:2], ga[:, 0:2])
                s = 1 << (k - 1)
                vo = lev[k][:].bitcast(bf16)
                vi = lev[k - 1][:].bitcast(bf16)
                # wavefront: each level builds in 4 pieces; piece j only
                # needs the previous level's pieces j and j+1 (range-level
                # dependency tracking), so levels pipeline diagonally
                QB = ([0, 318, 638, 958, HN[1]] if k == 1
                      else [0, 320, 640, 960, HN[k]])
                for a, b in zip(QB, QB[1:]):
                    nc.vector.tensor_tensor(
                        vo[:, 16 * a:16 * b],
                        vi[:, 16 * a:16 * b],
                        vi[:, 16 * (a + s):16 * (b + s)], MAX)
                do_gathers(k)
                if k >= 6:
                    # finals for the previous class, issued after this
                    # level's build pieces so builds never queue behind them
                    flush_finals()
            flush_finals()
    nc.compile()
    return nc


def _f32_to_bf16_u16(x):
    u = x.astype(np.float32).view(np.uint32)
    rounded = u + 0x7FFF + ((u >> 16) & 1)
    return (rounded >> 16).astype(np.uint16)


def _queries(segments, max_len):
    seg = np.clip(np.asarray(segments, np.float32)[0], 0.0,
                  np.float32(max_len - 1))
    fams = []
    for f in (0, 1):
        lo = np.floor(seg[:, 2 * f]).astype(np.int64)
        hi = np.ceil(seg[:, 2 * f + 1]).astype(np.int64)
        hi = np.maximum(hi, lo + 1)
        ln = hi - lo
        k = np.floor(np.log2(ln.astype(np.float64))).astype(np.int64)
        fams.append((k, lo, hi - (1 << k)))
    return fams


def _layout(fams):
    """Chunk layout unified across the four (family, half) streams."""
    streams = []
    for f in (0, 1):
        k, p1, p2 = fams[f]
        for h in (0, 1):
            m = (k >= 5) & (k <= 7) & ((p1 >= 1024) == bool(h))
            streams.append((f, h, k, p1, p2, m))
    counts = {g: 0 for g in GROUPS}
    for _, _, k, _, _, m in streams:
        for g in GROUPS:
            counts[g] = max(counts[g], int(np.sum(m & (k == g))))
    chunks = _plan_chunks(counts)
    qtot = sum(c for _, c in chunks)

    lay = {}
    for f, h, k, p1, p2, m in streams:
        tsort = {g: np.nonzero(m & (k == g))[0] for g in GROUPS}
        used = {g: 0 for g in GROUPS}
        idx_stream, perm = [], []
        for g, ch in chunks:
            ts = tsort[g][used[g]:used[g] + ch]
            used[g] += ch
            npad = ch - len(ts)
            a = np.concatenate([p1[ts] - H_OFF[h],
                                np.zeros(npad, np.int64)])
            b = np.concatenate([p2[ts] - H_OFF[h],
                                np.zeros(npad, np.int64)])
            assert (a >= 0).all() and (b >= 0).all()
            assert (a < HN[g]).all() if len(ts) else True
            idx_stream.append(np.concatenate([a, b]))
            perm.append(np.concatenate([ts, -np.ones(npad, np.int64)]))
        idx = np.concatenate(idx_stream).astype(np.int16)
        assert idx.size == 2 * qtot
        wrapped = np.tile(idx.reshape(-1, 16).T, (8, 1)).astype(np.int16)
        lay[(f, h)] = (wrapped, np.concatenate(perm))
    k0s = []
    for f in (0, 1):
        k, p1, p2 = fams[f]
        th = np.nonzero((k < 5) | (k == 8))[0]
        # reconstruct hi = p2 + 2^k
        k0s.append((th, p1[th], p2[th] + (1 << k[th].astype(np.int64))))
    return chunks, lay, k0s


def prepare(feature, segments, max_len):
    feature = np.asarray(feature, np.float32)
    u16 = _f32_to_bf16_u16(feature)           # [B, C2, T]
    fams = _queries(segments, int(max_len))
    chunks, lay, k0s = _layout(fams)
    in_maps, perms = [], []
    for c in range(N_CORES):
        f, h, i = c // 4, (c // 2) % 2, c % 2
        # lanes j = 0..15: batch 4i + j//4, channels 512f + 128*(j%4) + p
        x = u16[4 * i:4 * i + 4, 512 * f:512 * (f + 1),
                H_OFF[h]:H_OFF[h] + HT]                    # [4,512,HT]
        x = x.reshape(4, 4, P, HT).transpose(2, 3, 0, 1)   # [p,e,b,cb]
        packed = np.ascontiguousarray(x.reshape(P, HT, 16)).view(np.uint32)
        packed = packed.reshape(P, 8 * HT).astype(np.int32, copy=False)
        wrapped, perm = lay[(f, h)]
        in_maps.append({"feat": packed, "idxw": wrapped})
        perms.append(perm)
    return chunks, in_maps, perms, k0s


def postprocess(results, perms, k0s, feature):
    feature = np.asarray(feature, np.float32)
    out = np.empty((B, C2, T), np.float32)
    for c in range(N_CORES):
        f, h, i = c // 4, (c // 2) % 2, c % 2
        r = np.asarray(results[c]["out"])          # [P, 8*qtot] i32
        qtot = r.shape[1] // 8
        u16 = r.view(np.uint16).reshape(P, qtot, 16)
        perm = perms[c]
        valid = perm >= 0
        tq = perm[valid]
        v = u16[:, valid, :]                       # [P, nq, 16]
        f32 = (v.astype(np.uint32) << 16).view(np.float32)
        f32 = f32.transpose(2, 0, 1).reshape(4, 4, P, -1).reshape(4, 512, -1)
        out[4 * i:4 * i + 4, 512 * f:512 * (f + 1), :][:, :, tq] = f32
    for f in (0, 1):
        th, loh, hih = k0s[f]
        ch = slice(512 * f, 512 * (f + 1))
        for t, lo, hi in zip(th, loh, hih):
            out[:, ch, t] = feature[:, ch, lo:hi].max(axis=-1)
    return out


def kernel(feature, segments, max_len=T, **_unused):
    from concourse import bass_utils

    feature = np.asarray(feature, dtype=np.float32)
    assert feature.shape == (B, C2, T), feature.shape
    chunks, in_maps, perms, k0s = prepare(feature, segments, int(max_len))

    key = tuple(chunks)
    if _CACHE.get("key") != key:
        _CACHE["nc"] = _build_program(chunks)
        _CACHE["key"] = key
    nc = _CACHE["nc"]

    res = bass_utils.run_bass_kernel_spmd(
        nc, in_maps, core_ids=list(range(N_CORES)))
    return postprocess(res.results, perms, k0s, feature)
